# revision 3
# baseline (speedup 1.0000x reference)
"""Trainium2 Bass kernel for nn_Confidence_Score (gnn_message_passing).

Math: with S_g = sum of x over nodes of graph g and n_g = node count,
every node of graph g has identical output:
    h1_g = relu(S_g @ W1 + b1);  h2_g = relu((n_g*h1_g) @ W2 + b2)
    c_g  = h2_g @ Wc + bc;  out = sp/(1+sp), sp = softplus(c_g)

Kernel design (per core, nodes sharded graph-aligned):
  Pass 1: batch is SORTED, so every 128-node chunk spans <=2 consecutive
  graphs. Each chunk does ONE matmul: stationary = x chunk [128n x 128d]
  (fp8, FWL fast weight load), moving = host-built one-hot window mask
  [128n x W] (fp8, W~4), accumulating S_T[d, g] into PSUM columns
  [w_c, w_c+W) -- a compile-time per-chunk window schedule shared by all
  cores. No vector one-hot generation, no transposes.
  MLP: fully transposed orientation (graphs on the free axis); n_g and
  b1*n folded in via relu positive-homogeneity; biases pre-accumulated
  into PSUM by 1-row matmuls. softplus = relu(c) + ln(1+exp(-|c|)) with
  the exp/ln ACT table preloaded at t=0.
  Pass 2: nodes laid out [128 part x PJ] with <=1 graph boundary per
  partition row: out = og_a + (iota >= t) * (og_b - og_a), two DVE ops.
  og_a/og_b gathered from og by tiny one-hot matmuls.
"""

import os
import sys

for _p in ("/root/.axon_site", "/root/.axon_site/_ro/trn_rl_repo",
           "/root/.axon_site/_ro/pypackages", "/opt/trn_rl_repo"):
    if os.path.isdir(_p) and _p not in sys.path:
        sys.path.append(_p)

import numpy as np

N_CORES = 8
D = 128
H = 256
G_TOTAL = 512
G_PAD = 72
CHUNK = 128
X_SLICES = 10          # x DMA pipeline depth; CHUNK*X_SLICES node pad unit

_CACHE = {}


def _offsets(pj):
    o = {}
    o["W1"] = 0            # [128, 256]
    o["W2"] = 256          # [128, 512]: rows 0:128 at 256:512, 128:256 at 512:768
    o["WC"] = 768          # 2 cols
    o["OA"] = 770          # onehotA [72->128, 128]
    o["OB"] = 898          # onehotB
    o["IO"] = 1026         # iota [128, pj]
    r = 1026 + pj          # row-0 strips for 1-row bias matmuls
    o["B1"] = r            # [1, 256]
    o["B2"] = r + 256      # [1, 256]
    o["NR"] = r + 512      # [1, 72] graph node counts
    o["ONE"] = r + 584     # [1, 72] ones
    o["ZERO"] = r + 656    # [1, 128] zeros
    o["BC"] = r + 784      # [1, 1] bc
    o["END"] = r + 785
    return o


def _build(nodes_pad, wsched):
    """Compile the single-core program. wsched = (W, (w_0, w_1, ...))."""
    from contextlib import ExitStack

    import concourse.bacc as bacc
    import concourse.mybir as mybir
    import concourse.tile as tile

    f32 = mybir.dt.float32
    bf16 = mybir.dt.bfloat16
    fp8 = mybir.dt.float8e4
    AF = mybir.ActivationFunctionType
    OP = mybir.AluOpType

    W, wtab = wsched
    nch = nodes_pad // CHUNK
    pj = nodes_pad // 128
    assert nch % X_SLICES == 0
    OFF = _offsets(pj)
    WBW = OFF["END"]
    CFW = 2 + G_PAD

    nc = bacc.Bacc("TRN2", target_bir_lowering=False, debug=False)

    xs_d = nc.dram_tensor("xs", [128, nodes_pad], fp8, kind="ExternalInput").ap()
    mk_d = nc.dram_tensor("mk", [128, W * nch], fp8, kind="ExternalInput").ap()
    wb_d = nc.dram_tensor("wb", [128, WBW], bf16, kind="ExternalInput").ap()
    cf_d = nc.dram_tensor("cf", [128, CFW], f32, kind="ExternalInput").ap()
    out_d = nc.dram_tensor("out", [128, pj], f32, kind="ExternalOutput").ap()

    with tile.TileContext(nc) as tc, ExitStack() as ctx:
        const = ctx.enter_context(tc.tile_pool(name="const", bufs=1))
        xp = ctx.enter_context(tc.tile_pool(name="xp", bufs=1))
        wk = ctx.enter_context(tc.tile_pool(name="wk", bufs=1))
        ps = ctx.enter_context(tc.tile_pool(name="ps", bufs=1, space="PSUM"))

        wb = const.tile([128, WBW], bf16)
        cf = const.tile([128, CFW], f32)
        mk = const.tile([128, W * nch], fp8)
        nc.scalar.dma_start(wb[:], wb_d[:])
        nc.scalar.dma_start(cf[:], cf_d[:])
        nc.gpsimd.dma_start(mk[:], mk_d[:])

        # preload the exp/ln activation table while DMAs stream
        scr = wk.tile([1, 2], f32)
        nc.vector.memset(scr[:], 0.0)
        nc.scalar.activation(scr[:, 1:2], scr[:, 0:1], AF.Exp)
        nc.scalar.activation(scr[:, 0:1], scr[:, 1:2], AF.Ln)

        # x slices on the sync queue (in-order completion)
        sl = nodes_pad // X_SLICES
        xsb = []
        for s in range(X_SLICES):
            t = xp.tile([128, sl], fp8)
            nc.sync.dma_start(t[:], xs_d[:, s * sl:(s + 1) * sl])
            xsb.append(t)

        st_ps = ps.tile([128, G_PAD], f32)
        h1ps_a = ps.tile([128, G_PAD], f32)
        h1ps_b = ps.tile([128, G_PAD], f32)
        h2ps_a = ps.tile([128, G_PAD], f32)
        h2ps_b = ps.tile([128, G_PAD], f32)
        c_ps = ps.tile([G_PAD, 1], f32)
        oga_ps = ps.tile([128, 1], f32)
        ogb_ps = ps.tile([128, 1], f32)

        def row(off, w):
            return wb[0:1, off:off + w]

        # PSUM inits via 1-row matmuls (start=True resets; later MMs accum)
        nc.tensor.matmul(st_ps[:], lhsT=row(OFF["ZERO"], 128),
                         rhs=row(OFF["ZERO"], G_PAD), start=True, stop=False,
                         skip_group_check=True)
        nc.tensor.matmul(h1ps_a[:], lhsT=row(OFF["B1"], 128),
                         rhs=row(OFF["NR"], G_PAD), start=True, stop=False,
                         skip_group_check=True)
        nc.tensor.matmul(h1ps_b[:], lhsT=row(OFF["B1"] + 128, 128),
                         rhs=row(OFF["NR"], G_PAD), start=True, stop=False,
                         skip_group_check=True)
        nc.tensor.matmul(h2ps_a[:], lhsT=row(OFF["B2"], 128),
                         rhs=row(OFF["ONE"], G_PAD), start=True, stop=False,
                         skip_group_check=True)
        nc.tensor.matmul(h2ps_b[:], lhsT=row(OFF["B2"] + 128, 128),
                         rhs=row(OFF["ONE"], G_PAD), start=True, stop=False,
                         skip_group_check=True)
        nc.tensor.matmul(c_ps[:], lhsT=row(OFF["ONE"], G_PAD),
                         rhs=row(OFF["BC"], 1), start=True, stop=False,
                         skip_group_check=True)

        # ---- pass 1: segment-sum via windowed one-hot moving masks ----
        for c in range(nch):
            s = c // (nch // X_SLICES)
            j0 = (c % (nch // X_SLICES)) * CHUNK
            w0 = wtab[c]
            nc.tensor.matmul(
                st_ps[:, w0:w0 + W],
                lhsT=xsb[s][:, j0:j0 + CHUNK],
                rhs=mk[:, W * c:W * (c + 1)],
                start=False, stop=(c == nch - 1), skip_group_check=True,
            )

        # ---- MLP (transposed orientation, graphs on free axis) ----
        snt = wk.tile([128, G_PAD], bf16)
        nc.vector.tensor_tensor(snt[:], st_ps[:], cf[:, 2:2 + G_PAD], op=OP.mult)

        nc.tensor.matmul(h1ps_a[:], lhsT=wb[:, OFF["W1"]:OFF["W1"] + 128],
                         rhs=snt[:], start=False, stop=True,
                         skip_group_check=True)
        nc.tensor.matmul(h1ps_b[:], lhsT=wb[:, OFF["W1"] + 128:OFF["W1"] + 256],
                         rhs=snt[:], start=False, stop=True,
                         skip_group_check=True)
        h1r_a = wk.tile([128, G_PAD], bf16)
        h1r_b = wk.tile([128, G_PAD], bf16)
        nc.scalar.activation(h1r_a[:], h1ps_a[:], AF.Relu)
        nc.vector.tensor_scalar_max(h1r_b[:], h1ps_b[:], 0.0)

        w2q = OFF["W2"]
        nc.tensor.matmul(h2ps_a[:], lhsT=wb[:, w2q:w2q + 128],
                         rhs=h1r_a[:], start=False, stop=False,
                         skip_group_check=True)
        nc.tensor.matmul(h2ps_a[:], lhsT=wb[:, w2q + 256:w2q + 384],
                         rhs=h1r_b[:], start=False, stop=True,
                         skip_group_check=True)
        nc.tensor.matmul(h2ps_b[:], lhsT=wb[:, w2q + 128:w2q + 256],
                         rhs=h1r_a[:], start=False, stop=False,
                         skip_group_check=True)
        nc.tensor.matmul(h2ps_b[:], lhsT=wb[:, w2q + 384:w2q + 512],
                         rhs=h1r_b[:], start=False, stop=True,
                         skip_group_check=True)
        h2r_a = wk.tile([128, G_PAD], bf16)
        h2r_b = wk.tile([128, G_PAD], bf16)
        nc.scalar.activation(h2r_a[:], h2ps_a[:], AF.Relu)
        nc.vector.tensor_scalar_max(h2r_b[:], h2ps_b[:], 0.0)

        nc.tensor.matmul(c_ps[:], lhsT=h2r_a[:], rhs=wb[:, OFF["WC"]:OFF["WC"] + 1],
                         start=False, stop=False, skip_group_check=True)
        nc.tensor.matmul(c_ps[:], lhsT=h2r_b[:], rhs=wb[:, OFF["WC"] + 1:OFF["WC"] + 2],
                         start=False, stop=True, skip_group_check=True)

        # softplus: sp = relu(c) + ln(1 + exp(-|c|)); og = 1 - 1/(1+sp)
        rl1 = wk.tile([G_PAD, 1], f32)
        nc.vector.tensor_scalar(rl1[:], c_ps[:], 0.0, 1.0, op0=OP.max, op1=OP.add)
        ngc = wk.tile([G_PAD, 1], f32)
        nc.vector.tensor_scalar_mul(ngc[:], c_ps[:], -1.0)
        ab = wk.tile([G_PAD, 1], f32)
        nc.vector.tensor_tensor(ab[:], c_ps[:], ngc[:], op=OP.min)
        ex = wk.tile([G_PAD, 1], f32)
        nc.scalar.activation(ex[:], ab[:], AF.Exp)
        ex1 = wk.tile([G_PAD, 1], f32)
        nc.vector.tensor_scalar_add(ex1[:], ex[:], 1.0)
        lg = wk.tile([G_PAD, 1], f32)
        nc.scalar.activation(lg[:], ex1[:], AF.Ln)
        sp1 = wk.tile([G_PAD, 1], f32)
        nc.vector.tensor_tensor(sp1[:], rl1[:], lg[:], op=OP.add)
        rc = wk.tile([G_PAD, 1], f32)
        nc.vector.reciprocal(rc[:], sp1[:])
        og = wk.tile([G_PAD, 1], bf16)
        nc.vector.tensor_scalar(og[:], rc[:], -1.0, 1.0, op0=OP.mult, op1=OP.add)

        # ---- pass 2: per-partition two-graph select ----
        nc.tensor.matmul(oga_ps[:], lhsT=wb[0:G_PAD, OFF["OA"]:OFF["OA"] + 128],
                         rhs=og[:], start=True, stop=True)
        nc.tensor.matmul(ogb_ps[:], lhsT=wb[0:G_PAD, OFF["OB"]:OFF["OB"] + 128],
                         rhs=og[:], start=True, stop=True)
        oga = wk.tile([128, 1], f32)
        nc.vector.tensor_copy(oga[:], oga_ps[:])
        dg = wk.tile([128, 1], f32)
        nc.vector.tensor_tensor(dg[:], ogb_ps[:], oga[:], op=OP.subtract)
        tmp = wk.tile([128, pj], f32)
        nc.vector.tensor_scalar(tmp[:], wb[:, OFF["IO"]:OFF["IO"] + pj],
                                cf[:, 0:1], dg[:], op0=OP.is_ge, op1=OP.mult)
        outsb = wk.tile([128, pj], f32)
        nc.vector.tensor_scalar(outsb[:], tmp[:], oga[:], None, op0=OP.add)
        nc.sync.dma_start(out_d[:], outsb[:])

    nc.compile()
    return nc


def _shard(batch):
    """Graph-aligned split of nodes across cores, balanced by node count."""
    n = batch.shape[0]
    counts = np.bincount(batch, minlength=G_TOTAL).astype(np.int64)
    bounds = np.concatenate([[0], np.cumsum(counts)])
    gsplit = [0]
    for k in range(1, N_CORES):
        t = k * n // N_CORES
        g = int(np.searchsorted(bounds, t))
        if g > 0 and abs(int(bounds[g - 1]) - t) < abs(int(bounds[g]) - t):
            g -= 1
        g = min(max(g, gsplit[-1]), G_TOTAL)
        gsplit.append(g)
    gsplit.append(G_TOTAL)
    return counts, bounds, gsplit


def kernel(**inputs):
    import ml_dtypes
    from concourse.bass_utils import run_bass_kernel_spmd

    bf16 = ml_dtypes.bfloat16
    fp8 = ml_dtypes.float8_e4m3
    x = np.ascontiguousarray(np.asarray(inputs["x"], dtype=np.float32))
    batch = np.asarray(inputs["batch"]).astype(np.int64)
    W1 = np.asarray(inputs["W1"], dtype=np.float32)
    b1 = np.asarray(inputs["b1"], dtype=np.float32)
    W2 = np.asarray(inputs["W2"], dtype=np.float32)
    b2 = np.asarray(inputs["b2"], dtype=np.float32)
    Wc = np.asarray(inputs["Wc"], dtype=np.float32).reshape(H, 1)
    bc = np.asarray(inputs["bc"], dtype=np.float32).reshape(1)

    n = batch.shape[0]
    counts, bounds, gsplit = _shard(batch)
    node_cnt = [int(bounds[gsplit[k + 1]] - bounds[gsplit[k]]) for k in range(N_CORES)]
    pad_unit = CHUNK * X_SLICES
    nodes_pad = int(-(-max(node_cnt) // pad_unit) * pad_unit)
    nch = nodes_pad // CHUNK
    pj = nodes_pad // 128
    assert max(gsplit[k + 1] - gsplit[k] for k in range(N_CORES)) <= G_PAD - 1

    # local (per-core) batch ids, padded by repeating the last real id
    bt = np.zeros((N_CORES, nodes_pad), dtype=np.int64)
    for k in range(N_CORES):
        gs, ge = gsplit[k], gsplit[k + 1]
        ns, ne = int(bounds[gs]), int(bounds[ge])
        b = batch[ns:ne] - gs
        bt[k, :len(b)] = b
        bt[k, len(b):] = b[-1]

    # chunk graph windows, shared across cores
    seg = bt.reshape(N_CORES, nch, CHUNK)
    gl = seg.min(axis=2)   # [cores, nch]
    gh = seg.max(axis=2)
    assert int((gh - gl).max()) <= 1, "chunk spans >2 graphs"
    wtab = gl.min(axis=0)
    W = int((gh - wtab[None, :]).max()) + 1
    W = max(W, 2)
    assert wtab.max() + W <= G_PAD

    key = (nodes_pad, W, tuple(int(v) for v in wtab))
    if key not in _CACHE:
        _CACHE[key] = _build(nodes_pad, (W, tuple(int(v) for v in wtab)))
    nc = _CACHE[key]

    OFF = _offsets(pj)
    WBW = OFF["END"]

    # shared weight pack (bf16)
    wb0 = np.zeros((128, WBW), dtype=np.float32)
    wb0[:, 0:256] = W1
    wb0[:, 256:512] = W2[0:128]
    wb0[:, 512:768] = W2[128:256]
    wb0[:, OFF["WC"]] = Wc[0:128, 0]
    wb0[:, OFF["WC"] + 1] = Wc[128:256, 0]
    wb0[:, OFF["IO"]:OFF["IO"] + pj] = np.arange(pj, dtype=np.float32)[None, :]
    wb0[0, OFF["B1"]:OFF["B1"] + 256] = b1
    wb0[0, OFF["B2"]:OFF["B2"] + 256] = b2
    wb0[0, OFF["ONE"]:OFF["ONE"] + G_PAD] = 1.0
    wb0[0, OFF["BC"]] = bc[0]

    in_maps = []
    for k in range(N_CORES):
        gs, ge = gsplit[k], gsplit[k + 1]
        ns, ne = int(bounds[gs]), int(bounds[ge])
        cnt = ne - ns
        ng = ge - gs

        xp = np.zeros((nodes_pad, D), dtype=np.float32)
        xp[:cnt] = x[ns:ne]
        xs = np.ascontiguousarray(
            xp.reshape(nch, CHUNK, D).transpose(1, 0, 2)
        ).reshape(128, nodes_pad).astype(fp8)

        mk = np.zeros((128, W * nch), dtype=np.float32)
        bk = bt[k]
        p_idx = np.arange(nodes_pad) % CHUNK
        c_idx = np.arange(nodes_pad) // CHUNK
        off = bk - wtab[c_idx]
        assert off.min() >= 0 and off.max() < W, "window miss"
        real = np.arange(nodes_pad) < cnt
        mk[p_idx[real], W * c_idx[real] + off[real]] = 1.0
        mk = mk.astype(fp8)

        wbk = wb0.copy()
        wbk[0, OFF["NR"]:OFF["NR"] + ng] = counts[gs:ge].astype(np.float32)
        # pass-2 per-partition run structure
        seg2 = bk.reshape(128, pj)
        ga = seg2[:, 0]
        gb = seg2[:, -1]
        assert int((gb - ga).max()) <= 1, "partition spans >2 graphs"
        t = np.where(ga == gb, pj, np.argmax(seg2 == gb[:, None], axis=1))
        oa = np.zeros((128, 128), dtype=np.float32)
        ob = np.zeros((128, 128), dtype=np.float32)
        oa[ga, np.arange(128)] = 1.0
        ob[gb, np.arange(128)] = 1.0
        wbk[:, OFF["OA"]:OFF["OA"] + 128] = oa
        wbk[:, OFF["OB"]:OFF["OB"] + 128] = ob

        cfk = np.zeros((128, 2 + G_PAD), dtype=np.float32)
        cfk[:, 0] = t.astype(np.float32)
        cfk[:, 2:2 + ng] = counts[gs:ge].astype(np.float32)[None, :]

        in_maps.append({
            "xs": xs,
            "mk": np.ascontiguousarray(mk),
            "wb": np.ascontiguousarray(wbk.astype(bf16)),
            "cf": np.ascontiguousarray(cfk),
        })

    res = run_bass_kernel_spmd(nc, in_maps, core_ids=list(range(N_CORES)))
    outs = []
    for k in range(N_CORES):
        o = res.results[k]["out"].reshape(-1)
        outs.append(o[: node_cnt[k]])
    return np.concatenate(outs).reshape(n, 1).astype(np.float32)
